# revision 41
# baseline (speedup 1.0000x reference)
"""Trainium2 Bass kernel for AttentionWithKVCache.

Problem shapes (hardcoded): B=2, T=1024, C=2048, H=16, DK=128, S_CACHE=1024.
Reference computes:
    Q,K,V = x@W{q,k,v}.T ; K/V = concat(cache, K/V) on seq axis
    scores = QK^T/sqrt(dk) masked with tril(ones(T, S))  (no cache offset!)
    out = softmax(scores)@V @ Wo.T ; returns (out, K, V)

Because the mask is tril(ones(1024, 2048)), query t only attends s <= t,
which lies entirely inside the cache region (s < 1024). So attention reads
only cache_k/cache_v; the projected K/V only feed the returned K/V tensors.

Sharding: 8 cores = 2 batches x 4 head-groups (4 heads each). Each core:
  - projects Q/K/V for its 4 heads (columns of Wq/Wk/Wv)
  - causal attention over its heads against the (full) cache
  - partial output projection over its 512 columns of Wo
Host sums the 4 partial outputs per batch and assembles K/V (cache rows are
passed through on host in exact fp32).

Dataflow on device is fully "feature-major" (tokens on the free axis):
matmuls are bf16 (inputs pre-cast and pre-transposed on host), accumulation
fp32 in PSUM. exp runs on the scalar engine (scale=1/sqrt(dk) folded in)
with score columns trimmed to the causal region; the softmax denominator is
a pairwise add tree (Pool+DVE) plus a GpSimd partition all-reduce; the tril
diagonal mask is a host-sent [128,128] tile multiplied on GpSimd.
"""

import math

import numpy as np
import ml_dtypes

B, T, C, H, DK = 2, 1024, 2048, 16, 128
S_CACHE = 1024
N_CORES = 8
GROUPS = 4          # head-groups (cores per batch)
HPC = H // GROUPS   # heads per core = 4
M = HPC * DK        # per-core projected width = 512
P = 128             # partitions
TT = 512            # token tile (free dim / PSUM bank)
NT = T // TT        # 2 token tiles
NCC = C // P        # 16 contraction chunks over C
NSC = S_CACHE // P  # 8 cache s-chunks

BF16 = ml_dtypes.bfloat16

_PROGRAM = None


def _build_program():
    import concourse.tile as tile
    import concourse.mybir as mybir
    from concourse import bacc
    from concourse._compat import get_trn_type

    f32 = mybir.dt.float32
    bf16 = mybir.dt.bfloat16
    Exp = mybir.ActivationFunctionType.Exp

    nc = bacc.Bacc(
        get_trn_type() or "TRN2",
        target_bir_lowering=False,
        debug=False,
        num_devices=N_CORES,
    )

    # Per-core inputs, already laid out [partition, free...] and bf16-cast.
    xT = nc.declare_dram_parameter("xT", [P, NCC, T], bf16, isOutput=False)
    wqT = nc.declare_dram_parameter("wqT", [P, NCC, M], bf16, isOutput=False)
    wkT = nc.declare_dram_parameter("wkT", [P, NCC, M], bf16, isOutput=False)
    wvT = nc.declare_dram_parameter("wvT", [P, NCC, M], bf16, isOutput=False)
    woT = nc.declare_dram_parameter("woT", [P, HPC, C], bf16, isOutput=False)
    ktc = nc.declare_dram_parameter("ktc", [P, HPC, S_CACHE], bf16, isOutput=False)
    vc = nc.declare_dram_parameter("vc", [P, HPC * NSC, DK], bf16, isOutput=False)
    tri = nc.declare_dram_parameter("tri", [P, P], bf16, isOutput=False)

    outT = nc.declare_dram_parameter("outT", [P, NCC, T], f32, isOutput=True)
    kTn = nc.declare_dram_parameter("kTn", [P, HPC, T], f32, isOutput=True)
    vTn = nc.declare_dram_parameter("vTn", [P, HPC, T], f32, isOutput=True)

    inv_sqrt_dk = 1.0 / math.sqrt(DK)

    import concourse.bass_isa as bass_isa

    with tile.TileContext(nc) as tc:
        with (
            tc.tile_pool(name="const", bufs=1) as const,
            tc.tile_pool(name="sb_exp", bufs=3) as sb_exp,
            tc.tile_pool(name="sb_out", bufs=6) as sb_out,
            tc.tile_pool(name="sb_misc", bufs=6) as sb_misc,
        ):
            # ---- resident loads, interleaved by c-chunk so the first
            # projection's operands land together and compute starts early
            xt = const.tile([P, NCC, T], bf16)
            wq_t = const.tile([P, NCC, M], bf16)
            wk_t = const.tile([P, NCC, M], bf16)
            wv_t = const.tile([P, NCC, M], bf16)
            for c in range(NCC):
                nc.sync.dma_start(xt[:, c], xT[:, c])
                nc.sync.dma_start(wq_t[:, c], wqT[:, c])
            for c in range(NCC):
                nc.sync.dma_start(wk_t[:, c], wkT[:, c])
                nc.sync.dma_start(wv_t[:, c], wvT[:, c])
            ktc_t = const.tile([P, HPC, S_CACHE], bf16)
            for h in range(HPC):
                nc.sync.dma_start(ktc_t[:, h], ktc[:, h])
            vc_t = const.tile([P, HPC * NSC, DK], bf16)
            for h in range(HPC):
                nc.sync.dma_start(
                    vc_t[:, h * NSC:(h + 1) * NSC], vc[:, h * NSC:(h + 1) * NSC]
                )
            tri_t = const.tile([P, P], bf16)
            nc.sync.dma_start(tri_t[:], tri[:])
            wo_t = const.tile([P, HPC, C], bf16)
            for h in range(HPC):
                nc.sync.dma_start(wo_t[:, h], woT[:, h])

            qT_t = const.tile([P, HPC, T], bf16)
            attoT_t = const.tile([P, HPC, T], bf16)

            # One shared PSUM pool (all 8 banks, one tag) — slots rotate
            # across projection, attention, and output-projection tiles, so
            # phase transitions never stall on a pool-boundary WAR.
            with tc.tile_pool(name="psp", bufs=8, space="PSUM") as psp:
                def pstile(name):
                    return psp.tile([P, TT], f32, tag="ps", name=name)

                # ---- PE warmup: keep the PE busy (and ramping to full clock)
                # during the initial input-DMA window; results are never read.
                warm_sb = const.tile([P, TT], bf16)
                nc.gpsimd.memset(warm_sb[:], 0.0)
                warm_ps = pstile("warm")
                NWARM = 12
                for i in range(NWARM):
                    nc.tensor.matmul(
                        warm_ps[:, :256], lhsT=warm_sb[:, :P], rhs=warm_sb[:, :256],
                        start=(i == 0), stop=(i == NWARM - 1),
                    )

                # ---- Q/K/V projections: psum[m(128), t(512)] += wT_c^T @ xT_c.
                # Q runs c-outer with all 8 (m, t) accumulations live, so each
                # arriving c-chunk unlocks 8 matmuls (PE stays ahead of the
                # input DMAs). K/V (inputs resident by then) run c-inner so
                # PSUM slots rotate and attention can start grabbing banks.
                def project_wide(w_t, consume):
                    pss = [pstile(f"proj_{m}_{t}") for m in range(HPC) for t in range(NT)]
                    for c in range(NCC):
                        for m in range(HPC):
                            for t in range(NT):
                                nc.tensor.matmul(
                                    pss[m * NT + t],
                                    lhsT=w_t[:, c, m * P:(m + 1) * P],
                                    rhs=xt[:, c, t * TT:(t + 1) * TT],
                                    start=(c == 0),
                                    stop=(c == NCC - 1),
                                )
                    for m in range(HPC):
                        for t in range(NT):
                            consume(pss[m * NT + t], m, t)

                def project_one(w_t, consume, m, t):
                    ps = pstile(f"proj_{m}_{t}")
                    for c in range(NCC):
                        nc.tensor.matmul(
                            ps,
                            lhsT=w_t[:, c, m * P:(m + 1) * P],
                            rhs=xt[:, c, t * TT:(t + 1) * TT],
                            start=(c == 0),
                            stop=(c == NCC - 1),
                        )
                    consume(ps, m, t)

                def q_consume(ps, m, t):
                    nc.scalar.copy(out=qT_t[:, m, t * TT:(t + 1) * TT], in_=ps)

                def kv_consume(dram):
                    def f(ps, m, t):
                        s = sb_out.tile([P, TT], f32, tag="evac")
                        nc.vector.tensor_copy(out=s, in_=ps)
                        nc.sync.dma_start(dram[:, m, t * TT:(t + 1) * TT], s)
                    return f

                project_wide(wq_t, q_consume)

                # ---- attention: all (t, h) first, then output projections —
                # out-proj matmuls cover the PE while the later token tile's
                # exp/denominator chain runs on ACT/DVE/Pool.
                def attention(t, h):
                    jmax = (t + 1) * (TT // P)  # causal: s-chunks needed
                    # exp(scores) chunks for this (h, t), one wide tile
                    eg = sb_exp.tile([P, NSC, TT], bf16, tag="expg")
                    for j in range(jmax):
                        # cols < mc are fully masked (s > t for all s in
                        # chunk j); cols [mc, mc+P) are the tril diagonal
                        off = t * TT - j * P
                        mc = max(0, -off)
                        ps = pstile("scores")
                        nc.tensor.matmul(
                            ps[:, mc:],
                            lhsT=ktc_t[:, h, j * P:(j + 1) * P],
                            rhs=qT_t[:, h, t * TT + mc:(t + 1) * TT],
                            start=True,
                            stop=True,
                        )
                        nc.scalar.activation(
                            eg[:, j, mc:], ps[:, mc:], Exp, scale=inv_sqrt_dk
                        )
                        if mc > 0:
                            nc.gpsimd.memset(eg[:, j, :mc], 0.0)
                        if off < P:
                            nc.gpsimd.tensor_mul(
                                out=eg[:, j, mc:mc + P],
                                in0=eg[:, j, mc:mc + P],
                                in1=tri_t[:],
                            )

                    # softmax denominator: pairwise tree (first level on Pool,
                    # rest on DVE), then partition all-reduce on Pool,
                    # reciprocal on DVE
                    def tree(parts, lvl=0):
                        if len(parts) == 1:
                            return parts[0]
                        nxt = []
                        for i in range(0, len(parts), 2):
                            a = sb_misc.tile(
                                [P, TT], f32, tag="ladd", name=f"ladd{lvl}_{i}",
                            )
                            eng = nc.gpsimd if lvl == 0 else nc.vector
                            eng.tensor_add(out=a, in0=parts[i], in1=parts[i + 1])
                            nxt.append(a)
                        return tree(nxt, lvl + 1)

                    lpart = tree([eg[:, j] for j in range(jmax)])
                    recip = sb_misc.tile([P, TT], f32, tag="lall")
                    nc.gpsimd.partition_all_reduce(
                        recip, lpart, channels=P, reduce_op=bass_isa.ReduceOp.add
                    )
                    nc.vector.reciprocal(recip, recip)

                    ops = pstile("av")
                    for j in range(jmax):
                        off = t * TT - j * P
                        mc = max(0, -off)
                        nc.tensor.matmul(
                            ops[:, mc:],
                            lhsT=vc_t[:, h * NSC + j],
                            rhs=eg[:, j, mc:],
                            start=(j == 0),
                            stop=(j == jmax - 1),
                        )
                    nc.vector.tensor_mul(
                        out=attoT_t[:, h, t * TT:(t + 1) * TT], in0=ops, in1=recip
                    )

                def out_proj(t):
                    # psum[c(128), t(512)] += woT_h^T @ attoT_h
                    for co in range(NCC):
                        ps = pstile("op")
                        for h in range(HPC):
                            nc.tensor.matmul(
                                ps,
                                lhsT=wo_t[:, h, co * P:(co + 1) * P],
                                rhs=attoT_t[:, h, t * TT:(t + 1) * TT],
                                start=(h == 0),
                                stop=(h == HPC - 1),
                            )
                        s = sb_out.tile([P, TT], f32, tag="evac")
                        nc.any.tensor_copy(out=s, in_=ps)
                        nc.sync.dma_start(outT[:, co, t * TT:(t + 1) * TT], s)

                # K projection, then V projection with token-tile-0 attention
                # interleaved (its exp/denominator chain hides under V-proj
                # matmuls), then token-tile-1 attention + output projections.
                for m in range(HPC):
                    project_one(wk_t, kv_consume(kTn), m, 0)
                    project_one(wk_t, kv_consume(kTn), m, 1)
                    if m >= 2:
                        attention(0, m - 2)
                for m in range(HPC):
                    if m >= HPC - 2:
                        attention(0, m)
                    project_one(wv_t, kv_consume(vTn), m, 0)
                    attention(1, m)
                    project_one(wv_t, kv_consume(vTn), m, 1)
                out_proj(0)
                out_proj(1)

    nc.finalize()
    return nc


def _host_inputs(x, cache_k, cache_v, Wq, Wk, Wv, Wo):
    """Build the 8 per-core input maps (bf16, SBUF-ready layouts)."""
    xTb = np.ascontiguousarray(x.transpose(0, 2, 1)).astype(BF16)  # [B, C, T]
    in_maps = []
    # tril: tri[s, u] = 1 iff s <= u (the diagonal-strip mask)
    tri = np.tril(np.ones((P, P), dtype=np.float32)).T.astype(BF16)

    for core in range(N_CORES):
        b = core // GROUPS
        g = core % GROUPS
        h0 = g * HPC
        m0 = h0 * DK  # column offset into the 2048-wide projection space

        # xT: [P, NCC, T]; [p, c, t] = x[b, t, c*128+p]
        xT = np.ascontiguousarray(
            xTb[b].reshape(NCC, P, T).transpose(1, 0, 2))

        def wslice(W):
            # [P, NCC, M]; [p, c, m] = W[m0+m, c*128+p]
            w = W[m0:m0 + M, :].astype(BF16)          # [M, C]
            return np.ascontiguousarray(
                w.T.reshape(NCC, P, M).transpose(1, 0, 2))

        # woT: [P, HPC, C]; [p, h, c] = Wo[c, m0 + h*128 + p]
        wo = Wo[:, m0:m0 + M].astype(BF16)            # [C, M]
        woT = np.ascontiguousarray(
            wo.T.reshape(HPC, P, C).transpose(1, 0, 2))

        # ktc: [P, HPC, S]; [p, h, s] = cache_k[b, h0+h, s, p]
        ktc = np.ascontiguousarray(
            cache_k[b, h0:h0 + HPC].astype(BF16).transpose(2, 0, 1))

        # vc: [P, HPC*NSC, DK]; [p, h*NSC+sc, d] = cache_v[b, h0+h, sc*128+p, d]
        vcm = cache_v[b, h0:h0 + HPC].astype(BF16)     # [HPC, S, DK]
        vc = np.ascontiguousarray(
            vcm.reshape(HPC, NSC, P, DK).transpose(2, 0, 1, 3)
        ).reshape(P, HPC * NSC, DK)

        in_maps.append({
            "xT": xT,
            "wqT": wslice(Wq),
            "wkT": wslice(Wk),
            "wvT": wslice(Wv),
            "woT": woT,
            "ktc": ktc,
            "vc": np.ascontiguousarray(vc),
            "tri": tri,
        })
    return in_maps


def _assemble(results, x, cache_k, cache_v):
    out = np.zeros((B, T, C), dtype=np.float32)
    K = np.empty((B, H, S_CACHE + T, DK), dtype=np.float32)
    V = np.empty((B, H, S_CACHE + T, DK), dtype=np.float32)
    K[:, :, :S_CACHE] = cache_k
    V[:, :, :S_CACHE] = cache_v
    for core in range(N_CORES):
        b = core // GROUPS
        g = core % GROUPS
        h0 = g * HPC
        r = results[core]
        # outT [P, NCC, T] -> partial out [T, C]: out[t, c*128+p]
        out[b] += r["outT"].transpose(2, 1, 0).reshape(T, C)
        # kTn [P(d), HPC, T] -> K[b, h0+h, S_CACHE+t, d]
        K[b, h0:h0 + HPC, S_CACHE:] = r["kTn"].transpose(1, 2, 0)
        V[b, h0:h0 + HPC, S_CACHE:] = r["vTn"].transpose(1, 2, 0)
    return out, K, V


def _get_program():
    global _PROGRAM
    if _PROGRAM is None:
        _PROGRAM = _build_program()
    return _PROGRAM


def kernel(x, cache_k, cache_v, Wq, Wk, Wv, Wo):
    from concourse.bass_utils import run_bass_kernel_spmd

    x = np.asarray(x, dtype=np.float32)
    cache_k = np.asarray(cache_k, dtype=np.float32)
    cache_v = np.asarray(cache_v, dtype=np.float32)
    Wq = np.asarray(Wq, dtype=np.float32)
    Wk = np.asarray(Wk, dtype=np.float32)
    Wv = np.asarray(Wv, dtype=np.float32)
    Wo = np.asarray(Wo, dtype=np.float32)

    nc = _get_program()
    in_maps = _host_inputs(x, cache_k, cache_v, Wq, Wk, Wv, Wo)
    res = run_bass_kernel_spmd(nc, in_maps, list(range(N_CORES)))
    return _assemble(res.results, x, cache_k, cache_v)
